# revision 1
# baseline (speedup 1.0000x reference)
"""Cross-attention kernel for Trainium2 (8 NeuronCores, batch-parallel).

Math per batch b (reference semantics):
  q = queries[b].reshape(C, N).T + q_pos        # [N, C]
  k = keys[b].reshape(C, N).T + k_pos
  v = values[b].reshape(C, N).T                 # [N, C]
  out = softmax(q @ k.T / 16) @ v, returned as [C, N] (c-major)

Device layout (per core = one batch):
  All matmuls in f32r (TF32 mode, 1 PE cycle/row).  S is computed transposed
  (S^T[k, q]) so that exp(S^T) tiles are directly the rhs of the O matmul
  (O^T = V^T A^T) and the softmax denominator comes from a ones-column
  matmul -- no on-chip transposes anywhere.
"""

import numpy as np

import concourse.bass as bass
import concourse.tile as tile
import concourse.mybir as mybir
from concourse import bacc
from concourse.bass_utils import run_bass_kernel_spmd

P = 128          # partitions
C = 256          # qk/v channel dim
N = 4096         # sequence (64*64)
B = 8            # batch == n_cores
QW = 512         # query block width (max fp32-class matmul free dim)
NQB = N // QW    # 8 query blocks
NKO = N // P     # 32 key chunks
KPB = QW // P    # key chunks per K block tile
SCALE = 1.0 / 16.0  # 1/sqrt(C)

F32 = mybir.dt.float32
F32R = mybir.dt.float32r
AF = mybir.ActivationFunctionType

_NC_CACHE = None


def tf32_round(x: np.ndarray) -> np.ndarray:
    u = x.view(np.uint32)
    u = (u + np.uint32(0x1000)) & np.uint32(0xFFFFE000)
    return u.view(np.float32)


def build_nc():
    nc = bacc.Bacc(None, target_bir_lowering=False)
    qt = nc.dram_tensor("qt", [C, N], F32, kind="ExternalInput")
    kt = nc.dram_tensor("kt", [C, N], F32, kind="ExternalInput")
    v = nc.dram_tensor("v", [N, C], F32R, kind="ExternalInput")
    qp = nc.dram_tensor("qp", [C, N], F32, kind="ExternalInput")
    kp = nc.dram_tensor("kp", [C, N], F32, kind="ExternalInput")
    o = nc.dram_tensor("o", [C, N], F32, kind="ExternalOutput")

    qt3 = qt.rearrange("(co p) n -> p co n", p=P)
    kt3 = kt.rearrange("(co p) n -> p co n", p=P)
    qp3 = qp.rearrange("(co p) n -> p co n", p=P)
    kp3 = kp.rearrange("(co p) n -> p co n", p=P)
    v3 = v.rearrange("(ko p) c -> p ko c", p=P)

    with tile.TileContext(nc) as tc:
        with (
            tc.tile_pool(name="consts", bufs=1) as consts,
            tc.tile_pool(name="qk", bufs=NQB) as qk,
            tc.tile_pool(name="vp", bufs=NKO) as vp,
            tc.tile_pool(name="raw", bufs=3) as raw,
            tc.tile_pool(name="atp", bufs=6) as atp,
            tc.tile_pool(name="small", bufs=2) as small,
            tc.tile_pool(name="outp", bufs=2) as outp,
            tc.tile_pool(name="ps_s", bufs=2, space="PSUM") as ps_s,
            tc.tile_pool(name="ps_o", bufs=2, space="PSUM") as ps_o,
            tc.tile_pool(name="ps_r", bufs=1, space="PSUM") as ps_r,
            tc.tile_pool(name="ps_b", bufs=1, space="PSUM") as ps_b,
        ):
            ones_f = consts.tile([P, 2], F32, tag="ones_f")
            nc.vector.memset(ones_f, 1.0)
            ones_c = consts.tile([P, 2], F32R, tag="ones_c")
            nc.vector.tensor_copy(ones_c, ones_f)
            ones_rf = consts.tile([1, P], F32, tag="ones_rf")
            nc.vector.memset(ones_rf, 1.0)
            ones_r = consts.tile([1, P], F32R, tag="ones_r")
            nc.vector.tensor_copy(ones_r, ones_rf)

            # K blocks (pos-added, f32r) -- needed in full from the start.
            kblks = []
            for j in range(NQB):
                sl = slice(j * QW, (j + 1) * QW)
                kraw = raw.tile([P, 2, QW], F32, tag="kraw")
                kpos = raw.tile([P, 2, QW], F32, tag="kpos")
                nc.sync.dma_start(kraw, kt3[:, :, sl])
                nc.sync.dma_start(kpos, kp3[:, :, sl])
                kb = qk.tile([P, 2, QW], F32R, tag="kblk")
                nc.vector.tensor_add(kb, kraw, kpos)
                kblks.append(kb)

            # V chunks [k=128, c=256], f32r via host-side TF32 rounding.
            vcs = []
            for ko in range(NKO):
                vc = vp.tile([P, C], F32R, tag="v")
                nc.sync.dma_start(vc, v3[:, ko, :])
                vcs.append(vc)

            def emit_epilogue(j, po0, po1, pr):
                sl = slice(j * QW, (j + 1) * QW)
                inv = small.tile([1, QW], F32R, tag="inv")
                with nc.allow_low_precision(
                    reason="TF32 rounding of softmax reciprocal"
                ):
                    nc.vector.reciprocal(inv, pr[0:1, :])
                pb = ps_b.tile([P, QW], F32, tag="b")
                nc.tensor.matmul(pb, ones_r, inv, start=True, stop=True)
                bs = small.tile([P, QW], F32, tag="bs")
                nc.vector.tensor_copy(bs, pb)
                oo0 = outp.tile([P, QW], F32, tag="oo0")
                nc.vector.tensor_mul(oo0, po0, bs)
                nc.sync.dma_start(o[0:P, sl], oo0)
                oo1 = outp.tile([P, QW], F32, tag="oo1")
                nc.vector.tensor_mul(oo1, po1, bs)
                nc.sync.dma_start(o[P:C, sl], oo1)

            pending = None
            for j in range(NQB):
                sl = slice(j * QW, (j + 1) * QW)
                qraw = raw.tile([P, 2, QW], F32, tag="qraw")
                qpos = raw.tile([P, 2, QW], F32, tag="qpos")
                nc.sync.dma_start(qraw, qt3[:, :, sl])
                nc.sync.dma_start(qpos, qp3[:, :, sl])
                qb = qk.tile([P, 2, QW], F32R, tag="qblk")
                nc.vector.tensor_add(qb, qraw, qpos)

                po0 = ps_o.tile([P, QW], F32, tag="o0")
                po1 = ps_o.tile([P, QW], F32, tag="o1")
                pr = ps_r.tile([2, QW], F32, tag="r")

                prev_a = None
                for ko in range(NKO):
                    pss = ps_s.tile([P, QW], F32, tag="s")
                    jb, koff = divmod(ko, KPB)
                    for co in range(2):
                        nc.tensor.matmul(
                            pss,
                            kblks[jb][:, co, koff * P : (koff + 1) * P],
                            qb[:, co, :],
                            start=(co == 0),
                            stop=(co == 1),
                        )
                    a = atp.tile([P, QW], F32R, tag="a")
                    nc.scalar.activation(a, pss, AF.Exp, scale=SCALE)

                    if prev_a is not None:
                        pko = ko - 1
                        nc.tensor.matmul(po0, vcs[pko][:, 0:P], prev_a,
                                         start=(pko == 0), stop=False)
                        nc.tensor.matmul(po1, vcs[pko][:, P:C], prev_a,
                                         start=(pko == 0), stop=False)
                        nc.tensor.matmul(pr, ones_c, prev_a,
                                         start=(pko == 0), stop=False)
                    prev_a = a

                    if ko == 2 and pending is not None:
                        emit_epilogue(*pending)
                        pending = None

                # last chunk's O/R matmuls close the accumulation groups
                nc.tensor.matmul(po0, vcs[NKO - 1][:, 0:P], prev_a,
                                 start=False, stop=True)
                nc.tensor.matmul(po1, vcs[NKO - 1][:, P:C], prev_a,
                                 start=False, stop=True)
                nc.tensor.matmul(pr, ones_c, prev_a, start=False, stop=True)
                pending = (j, po0, po1, pr)

            emit_epilogue(*pending)

    nc.compile()
    return nc


def _get_nc():
    global _NC_CACHE
    if _NC_CACHE is None:
        _NC_CACHE = build_nc()
    return _NC_CACHE


def make_in_maps(queries, keys, values, q_pos_embedding, k_pos_embedding):
    queries = np.asarray(queries, dtype=np.float32)
    keys = np.asarray(keys, dtype=np.float32)
    values = np.asarray(values, dtype=np.float32)
    qpT = np.ascontiguousarray(
        np.asarray(q_pos_embedding, dtype=np.float32).reshape(N, C).T
    )
    kpT = np.ascontiguousarray(
        np.asarray(k_pos_embedding, dtype=np.float32).reshape(N, C).T
    )
    in_maps = []
    for b in range(B):
        vT = tf32_round(
            np.ascontiguousarray(values[b].reshape(C, N).T)
        )
        in_maps.append({
            "qt": np.ascontiguousarray(queries[b].reshape(C, N)),
            "kt": np.ascontiguousarray(keys[b].reshape(C, N)),
            "v": vT,
            "qp": qpT,
            "kp": kpT,
        })
    return in_maps


def kernel(queries, keys, values, q_pos_embedding, k_pos_embedding):
    nc = _get_nc()
    in_maps = make_in_maps(queries, keys, values, q_pos_embedding,
                           k_pos_embedding)
    res = run_bass_kernel_spmd(nc, in_maps, core_ids=list(range(B)))
    out = np.stack([r["o"].reshape(C, 64, 64) for r in res.results])
    return out.astype(np.float32)


# revision 6
# speedup vs baseline: 396.3186x; 396.3186x over previous
"""Cross-attention kernel for Trainium2 (8 NeuronCores, batch-parallel).

Math per batch b (reference semantics):
  q = queries[b].reshape(C, N).T + q_pos        # [N, C]
  k = keys[b].reshape(C, N).T + k_pos
  v = values[b].reshape(C, N).T                 # [N, C]
  out = softmax(q @ k.T / 16) @ v, returned as [C, N] (c-major)

Device layout (per core = one batch):
  All matmuls in f32r (TF32 mode, 1 PE cycle/row).  S is computed transposed
  (S^T[k, q]) so that exp(S^T) tiles are directly the rhs of the O matmul
  (O^T = V^T A^T) and the softmax denominator comes from a ones-column
  matmul -- no on-chip transposes anywhere.
"""

import numpy as np

import concourse.bass as bass
import concourse.tile as tile
import concourse.mybir as mybir
from concourse import bacc
from concourse.bass_utils import run_bass_kernel_spmd

P = 128          # partitions
C = 256          # qk/v channel dim
N = 4096         # sequence (64*64)
B = 8            # batch == n_cores
QW = 512         # query block width (max fp32-class matmul free dim)
NQB = N // QW    # 8 query blocks
NKO = N // P     # 32 key chunks
KPB = QW // P    # key chunks per K block tile
SCALE = 1.0 / 16.0  # 1/sqrt(C)

F32 = mybir.dt.float32
F32R = mybir.dt.float32r
AF = mybir.ActivationFunctionType

_NC_CACHE = None


def tf32_round(x: np.ndarray) -> np.ndarray:
    u = x.view(np.uint32)
    u = (u + np.uint32(0x1000)) & np.uint32(0xFFFFE000)
    return u.view(np.float32)


def build_nc(atp_bufs=6, raw_bufs=3, ps_s_bufs=3, ps_o_bufs=1):
    nc = bacc.Bacc(None, target_bir_lowering=False)
    qt = nc.dram_tensor("qt", [C, N], F32, kind="ExternalInput")
    kt = nc.dram_tensor("kt", [C, N], F32, kind="ExternalInput")
    v = nc.dram_tensor("v", [N, C], F32R, kind="ExternalInput")
    qp = nc.dram_tensor("qp", [C, N], F32, kind="ExternalInput")
    kp = nc.dram_tensor("kp", [C, N], F32, kind="ExternalInput")
    o = nc.dram_tensor("o", [C, N], F32, kind="ExternalOutput")

    qt3 = qt.rearrange("(co p) n -> p co n", p=P)
    kt3 = kt.rearrange("(co p) n -> p co n", p=P)
    qp3 = qp.rearrange("(co p) n -> p co n", p=P)
    kp3 = kp.rearrange("(co p) n -> p co n", p=P)
    v3 = v.rearrange("(ko p) c -> p ko c", p=P)

    with tile.TileContext(nc) as tc:
        with (
            tc.tile_pool(name="consts", bufs=1) as consts,
            tc.tile_pool(name="qk", bufs=NQB) as qk,
            tc.tile_pool(name="vp", bufs=NKO) as vp,
            tc.tile_pool(name="raw", bufs=raw_bufs) as raw,
            tc.tile_pool(name="atp", bufs=atp_bufs) as atp,
            tc.tile_pool(name="small", bufs=2) as small,
            tc.tile_pool(name="outp", bufs=2) as outp,
            tc.tile_pool(name="ps_s", bufs=ps_s_bufs, space="PSUM") as ps_s,
            tc.tile_pool(name="ps_o", bufs=ps_o_bufs, space="PSUM") as ps_o,
            tc.tile_pool(name="ps_r", bufs=1, space="PSUM") as ps_r,
            tc.tile_pool(name="ps_b", bufs=1, space="PSUM") as ps_b,
        ):
            ones_f = consts.tile([P, 2], F32, tag="ones_f")
            nc.vector.memset(ones_f, 1.0)
            ones_c = consts.tile([P, 2], F32R, tag="ones_c")
            nc.vector.tensor_copy(ones_c, ones_f)
            ones_rf = consts.tile([1, P], F32, tag="ones_rf")
            nc.vector.memset(ones_rf, 1.0)
            ones_r = consts.tile([1, P], F32R, tag="ones_r")
            nc.vector.tensor_copy(ones_r, ones_rf)

            # K blocks (pos-added, f32r) and V chunks, emitted in deadline
            # order: block 0's dependencies first (K0, V0..3), then K(jb)
            # interleaved with the V chunks needed just before it.
            def load_kblk(j):
                sl = slice(j * QW, (j + 1) * QW)
                kraw = raw.tile([P, 2, QW], F32, tag="kraw")
                kpos = raw.tile([P, 2, QW], F32, tag="kpos")
                nc.sync.dma_start(kraw, kt3[:, :, sl])
                nc.sync.dma_start(kpos, kp3[:, :, sl])
                kb = qk.tile([P, 2, QW], F32R, tag="kblk")
                nc.vector.tensor_add(kb, kraw, kpos)
                return kb

            def load_vchunk(ko):
                vc = vp.tile([P, C], F32R, tag="v")
                nc.sync.dma_start(vc, v3[:, ko, :])
                return vc

            kblks = {}
            vcs = {}
            kblks[0] = load_kblk(0)

            def emit_epilogue(j, po0, po1, pr):
                sl = slice(j * QW, (j + 1) * QW)
                inv = small.tile([1, QW], F32R, tag="inv")
                with nc.allow_low_precision(
                    reason="TF32 rounding of softmax reciprocal"
                ):
                    nc.vector.reciprocal(inv, pr[0:1, :])
                pb = ps_b.tile([P, QW], F32, tag="b")
                nc.tensor.matmul(pb, ones_r, inv, start=True, stop=True)
                bs = small.tile([P, QW], F32, tag="bs")
                nc.vector.tensor_copy(bs, pb)
                oo0 = outp.tile([P, QW], F32, tag="oo0")
                nc.vector.tensor_mul(oo0, po0, bs)
                nc.sync.dma_start(o[0:P, sl], oo0)
                oo1 = outp.tile([P, QW], F32, tag="oo1")
                nc.vector.tensor_mul(oo1, po1, bs)
                nc.sync.dma_start(o[P:C, sl], oo1)

            pending = None
            for j in range(NQB):
                sl = slice(j * QW, (j + 1) * QW)
                qraw = raw.tile([P, 2, QW], F32, tag="qraw")
                qpos = raw.tile([P, 2, QW], F32, tag="qpos")
                nc.sync.dma_start(qraw, qt3[:, :, sl])
                nc.sync.dma_start(qpos, qp3[:, :, sl])
                qb = qk.tile([P, 2, QW], F32R, tag="qblk")
                nc.vector.tensor_add(qb, qraw, qpos)

                if j == 0:
                    # deadline-ordered remaining loads: V(4jb..) then K(jb+1)
                    for jb in range(NQB):
                        for ko in range(4 * jb, 4 * jb + 4):
                            vcs[ko] = load_vchunk(ko)
                        if jb + 1 < NQB:
                            kblks[jb + 1] = load_kblk(jb + 1)

                po0 = ps_o.tile([P, QW], F32, tag="o0")
                po1 = ps_o.tile([P, QW], F32, tag="o1")
                pr = ps_r.tile([2, QW], F32, tag="r")

                prev_a = None
                for ko in range(NKO):
                    pss = ps_s.tile([P, QW], F32, tag="s")
                    jb, koff = divmod(ko, KPB)
                    for co in range(2):
                        nc.tensor.matmul(
                            pss,
                            kblks[jb][:, co, koff * P : (koff + 1) * P],
                            qb[:, co, :],
                            start=(co == 0),
                            stop=(co == 1),
                        )
                    a = atp.tile([P, QW], F32R, tag="a")
                    nc.scalar.activation(a, pss, AF.Exp, scale=SCALE)

                    if prev_a is not None:
                        pko = ko - 1
                        nc.tensor.matmul(po0, vcs[pko][:, 0:P], prev_a,
                                         start=(pko == 0), stop=False)
                        nc.tensor.matmul(po1, vcs[pko][:, P:C], prev_a,
                                         start=(pko == 0), stop=False)
                        nc.tensor.matmul(pr, ones_c, prev_a,
                                         start=(pko == 0), stop=False)
                    prev_a = a

                    if ko == 2 and pending is not None:
                        emit_epilogue(*pending)
                        pending = None

                # last chunk's O/R matmuls close the accumulation groups
                nc.tensor.matmul(po0, vcs[NKO - 1][:, 0:P], prev_a,
                                 start=False, stop=True)
                nc.tensor.matmul(po1, vcs[NKO - 1][:, P:C], prev_a,
                                 start=False, stop=True)
                nc.tensor.matmul(pr, ones_c, prev_a, start=False, stop=True)
                pending = (j, po0, po1, pr)

            emit_epilogue(*pending)

    nc.compile()
    return nc


def _get_nc():
    global _NC_CACHE
    if _NC_CACHE is None:
        _NC_CACHE = build_nc()
    return _NC_CACHE


def make_in_maps(queries, keys, values, q_pos_embedding, k_pos_embedding):
    queries = np.asarray(queries, dtype=np.float32)
    keys = np.asarray(keys, dtype=np.float32)
    values = np.asarray(values, dtype=np.float32)
    qpT = np.ascontiguousarray(
        np.asarray(q_pos_embedding, dtype=np.float32).reshape(N, C).T
    )
    kpT = np.ascontiguousarray(
        np.asarray(k_pos_embedding, dtype=np.float32).reshape(N, C).T
    )
    in_maps = []
    for b in range(B):
        vT = tf32_round(
            np.ascontiguousarray(values[b].reshape(C, N).T)
        )
        in_maps.append({
            "qt": np.ascontiguousarray(queries[b].reshape(C, N)),
            "kt": np.ascontiguousarray(keys[b].reshape(C, N)),
            "v": vT,
            "qp": qpT,
            "kp": kpT,
        })
    return in_maps


def kernel(queries, keys, values, q_pos_embedding, k_pos_embedding):
    nc = _get_nc()
    in_maps = make_in_maps(queries, keys, values, q_pos_embedding,
                           k_pos_embedding)
    res = run_bass_kernel_spmd(nc, in_maps, core_ids=list(range(B)))
    out = np.stack([r["o"].reshape(C, 64, 64) for r in res.results])
    return out.astype(np.float32)


def build_nc_trivial():
    """Same I/O signature, minimal work: used by test.py to subtract the
    per-call transfer/dispatch overhead from wall-clock timing."""
    nc = bacc.Bacc(None, target_bir_lowering=False)
    qt = nc.dram_tensor("qt", [C, N], F32, kind="ExternalInput")
    kt = nc.dram_tensor("kt", [C, N], F32, kind="ExternalInput")
    v = nc.dram_tensor("v", [N, C], F32R, kind="ExternalInput")
    qp = nc.dram_tensor("qp", [C, N], F32, kind="ExternalInput")
    kp = nc.dram_tensor("kp", [C, N], F32, kind="ExternalInput")
    o = nc.dram_tensor("o", [C, N], F32, kind="ExternalOutput")
    with tile.TileContext(nc) as tc:
        with tc.tile_pool(name="sb", bufs=2) as sb:
            t = sb.tile([P, 2, N], F32, tag="t")
            nc.sync.dma_start(t, qt.rearrange("(co p) n -> p co n", p=P))
            nc.sync.dma_start(o.rearrange("(co p) n -> p co n", p=P), t)
    nc.compile()
    return nc


# revision 8
# speedup vs baseline: 403.8298x; 1.0190x over previous
"""Cross-attention kernel for Trainium2 (8 NeuronCores, batch-parallel).

Math per batch b (reference semantics):
  q = queries[b].reshape(C, N).T + q_pos        # [N, C]
  k = keys[b].reshape(C, N).T + k_pos
  v = values[b].reshape(C, N).T                 # [N, C]
  out = softmax(q @ k.T / 16) @ v, returned as [C, N] (c-major)

Device layout (per core = one batch):
  All matmuls in f32r (TF32 mode, 1 PE cycle/row).  S is computed transposed
  (S^T[k, q]) so that exp(S^T) tiles are directly the rhs of the O matmul
  (O^T = V^T A^T) and the softmax denominator comes from a ones-column
  matmul -- no on-chip transposes anywhere.
"""

import numpy as np

import concourse.bass as bass
import concourse.tile as tile
import concourse.mybir as mybir
from concourse import bacc
from concourse.bass_utils import run_bass_kernel_spmd

P = 128          # partitions
C = 256          # qk/v channel dim
N = 4096         # sequence (64*64)
B = 8            # batch == n_cores
QW = 512         # query block width (max fp32-class matmul free dim)
NQB = N // QW    # 8 query blocks
NKO = N // P     # 32 key chunks
KPB = QW // P    # key chunks per K block tile
SCALE = 1.0 / 16.0  # 1/sqrt(C)

F32 = mybir.dt.float32
F32R = mybir.dt.float32r
AF = mybir.ActivationFunctionType

_NC_CACHE = None


def tf32_round(x: np.ndarray) -> np.ndarray:
    u = x.view(np.uint32)
    u = (u + np.uint32(0x1000)) & np.uint32(0xFFFFE000)
    return u.view(np.float32)


def build_nc(atp_bufs=6, raw_bufs=3, ps_s_bufs=4, ps_o_bufs=1, lag=3):
    nc = bacc.Bacc(None, target_bir_lowering=False)
    qt = nc.dram_tensor("qt", [C, N], F32, kind="ExternalInput")
    kt = nc.dram_tensor("kt", [C, N], F32, kind="ExternalInput")
    v = nc.dram_tensor("v", [N, C], F32R, kind="ExternalInput")
    qp = nc.dram_tensor("qp", [C, N], F32, kind="ExternalInput")
    kp = nc.dram_tensor("kp", [C, N], F32, kind="ExternalInput")
    o = nc.dram_tensor("o", [C, N], F32, kind="ExternalOutput")

    qt3 = qt.rearrange("(co p) n -> p co n", p=P)
    kt3 = kt.rearrange("(co p) n -> p co n", p=P)
    qp3 = qp.rearrange("(co p) n -> p co n", p=P)
    kp3 = kp.rearrange("(co p) n -> p co n", p=P)
    v3 = v.rearrange("(ko p) c -> p ko c", p=P)

    with tile.TileContext(nc) as tc:
        with (
            tc.tile_pool(name="consts", bufs=1) as consts,
            tc.tile_pool(name="qk", bufs=NQB) as qk,
            tc.tile_pool(name="vp", bufs=NKO) as vp,
            tc.tile_pool(name="raw", bufs=raw_bufs) as raw,
            tc.tile_pool(name="atp", bufs=atp_bufs) as atp,
            tc.tile_pool(name="small", bufs=2) as small,
            tc.tile_pool(name="outp", bufs=2) as outp,
            tc.tile_pool(name="ps_s", bufs=ps_s_bufs, space="PSUM") as ps_s,
            tc.tile_pool(name="ps_o", bufs=ps_o_bufs, space="PSUM") as ps_o,
            tc.tile_pool(name="ps_r", bufs=1, space="PSUM") as ps_r,
            tc.tile_pool(name="ps_b", bufs=1, space="PSUM") as ps_b,
        ):
            ones_f = consts.tile([P, 2], F32, tag="ones_f")
            nc.vector.memset(ones_f, 1.0)
            ones_c = consts.tile([P, 2], F32R, tag="ones_c")
            nc.vector.tensor_copy(ones_c, ones_f)
            ones_rf = consts.tile([1, P], F32, tag="ones_rf")
            nc.vector.memset(ones_rf, 1.0)
            ones_r = consts.tile([1, P], F32R, tag="ones_r")
            nc.vector.tensor_copy(ones_r, ones_rf)

            # K blocks (pos-added, f32r) and V chunks, emitted in deadline
            # order: block 0's dependencies first (K0, V0..3), then K(jb)
            # interleaved with the V chunks needed just before it.
            def load_kblk(j):
                sl = slice(j * QW, (j + 1) * QW)
                kraw = raw.tile([P, 2, QW], F32, tag="kraw")
                kpos = raw.tile([P, 2, QW], F32, tag="kpos")
                nc.sync.dma_start(kraw, kt3[:, :, sl])
                nc.sync.dma_start(kpos, kp3[:, :, sl])
                kb = qk.tile([P, 2, QW], F32R, tag="kblk")
                nc.vector.tensor_add(kb, kraw, kpos)
                return kb

            def load_vchunk(ko):
                vc = vp.tile([P, C], F32R, tag="v")
                nc.sync.dma_start(vc, v3[:, ko, :])
                return vc

            kblks = {}
            vcs = {}
            kblks[0] = load_kblk(0)

            def emit_epilogue(j, po0, po1, pr):
                sl = slice(j * QW, (j + 1) * QW)
                inv = small.tile([1, QW], F32R, tag="inv")
                with nc.allow_low_precision(
                    reason="TF32 rounding of softmax reciprocal"
                ):
                    nc.vector.reciprocal(inv, pr[0:1, :])
                pb = ps_b.tile([P, QW], F32, tag="b")
                nc.tensor.matmul(pb, ones_r, inv, start=True, stop=True)
                bs = small.tile([P, QW], F32, tag="bs")
                nc.vector.tensor_copy(bs, pb)
                oo0 = outp.tile([P, QW], F32, tag="oo0")
                nc.vector.tensor_mul(oo0, po0, bs)
                nc.sync.dma_start(o[0:P, sl], oo0)
                oo1 = outp.tile([P, QW], F32, tag="oo1")
                nc.vector.tensor_mul(oo1, po1, bs)
                nc.sync.dma_start(o[P:C, sl], oo1)

            pending = None
            for j in range(NQB):
                sl = slice(j * QW, (j + 1) * QW)
                qraw = raw.tile([P, 2, QW], F32, tag="qraw")
                qpos = raw.tile([P, 2, QW], F32, tag="qpos")
                nc.sync.dma_start(qraw, qt3[:, :, sl])
                nc.sync.dma_start(qpos, qp3[:, :, sl])
                qb = qk.tile([P, 2, QW], F32R, tag="qblk")
                nc.vector.tensor_add(qb, qraw, qpos)

                if j == 0:
                    # deadline-ordered remaining loads: V(4jb..) then K(jb+1)
                    for jb in range(NQB):
                        for ko in range(4 * jb, 4 * jb + 4):
                            vcs[ko] = load_vchunk(ko)
                        if jb + 1 < NQB:
                            kblks[jb + 1] = load_kblk(jb + 1)

                po0 = ps_o.tile([P, QW], F32, tag="o0")
                po1 = ps_o.tile([P, QW], F32, tag="o1")
                pr = ps_r.tile([2, QW], F32, tag="r")

                a_q = {}

                for ko in range(NKO):
                    pss = ps_s.tile([P, QW], F32, tag="s")
                    jb, koff = divmod(ko, KPB)
                    for co in range(2):
                        nc.tensor.matmul(
                            pss,
                            kblks[jb][:, co, koff * P : (koff + 1) * P],
                            qb[:, co, :],
                            start=(co == 0),
                            stop=(co == 1),
                        )
                    a = atp.tile([P, QW], F32R, tag="a")
                    nc.scalar.activation(a, pss, AF.Exp, scale=SCALE)
                    a_q[ko] = a

                    if ko >= lag:
                        pko = ko - lag
                        av = a_q[pko]
                        nc.tensor.matmul(po0, vcs[pko][:, 0:P], av,
                                         start=(pko == 0), stop=False)
                        nc.tensor.matmul(po1, vcs[pko][:, P:C], av,
                                         start=(pko == 0), stop=False)
                        nc.tensor.matmul(pr, ones_c, av,
                                         start=(pko == 0), stop=False)
                        del a_q[pko]

                    if ko == 2 and pending is not None:
                        emit_epilogue(*pending)
                        pending = None

                # drain remaining lagged chunks; last closes the groups
                for pko in range(NKO - lag, NKO):
                    av = a_q[pko]
                    last = pko == NKO - 1
                    nc.tensor.matmul(po0, vcs[pko][:, 0:P], av,
                                     start=False, stop=last)
                    nc.tensor.matmul(po1, vcs[pko][:, P:C], av,
                                     start=False, stop=last)
                    nc.tensor.matmul(pr, ones_c, av, start=False, stop=last)
                    del a_q[pko]
                pending = (j, po0, po1, pr)

            emit_epilogue(*pending)

    nc.compile()
    return nc


def _get_nc():
    global _NC_CACHE
    if _NC_CACHE is None:
        _NC_CACHE = build_nc()
    return _NC_CACHE


def make_in_maps(queries, keys, values, q_pos_embedding, k_pos_embedding):
    queries = np.asarray(queries, dtype=np.float32)
    keys = np.asarray(keys, dtype=np.float32)
    values = np.asarray(values, dtype=np.float32)
    qpT = np.ascontiguousarray(
        np.asarray(q_pos_embedding, dtype=np.float32).reshape(N, C).T
    )
    kpT = np.ascontiguousarray(
        np.asarray(k_pos_embedding, dtype=np.float32).reshape(N, C).T
    )
    in_maps = []
    for b in range(B):
        vT = tf32_round(
            np.ascontiguousarray(values[b].reshape(C, N).T)
        )
        in_maps.append({
            "qt": np.ascontiguousarray(queries[b].reshape(C, N)),
            "kt": np.ascontiguousarray(keys[b].reshape(C, N)),
            "v": vT,
            "qp": qpT,
            "kp": kpT,
        })
    return in_maps


def kernel(queries, keys, values, q_pos_embedding, k_pos_embedding):
    nc = _get_nc()
    in_maps = make_in_maps(queries, keys, values, q_pos_embedding,
                           k_pos_embedding)
    res = run_bass_kernel_spmd(nc, in_maps, core_ids=list(range(B)))
    out = np.stack([r["o"].reshape(C, 64, 64) for r in res.results])
    return out.astype(np.float32)


def build_nc_trivial():
    """Same I/O signature, minimal work: used by test.py to subtract the
    per-call transfer/dispatch overhead from wall-clock timing."""
    nc = bacc.Bacc(None, target_bir_lowering=False)
    qt = nc.dram_tensor("qt", [C, N], F32, kind="ExternalInput")
    kt = nc.dram_tensor("kt", [C, N], F32, kind="ExternalInput")
    v = nc.dram_tensor("v", [N, C], F32R, kind="ExternalInput")
    qp = nc.dram_tensor("qp", [C, N], F32, kind="ExternalInput")
    kp = nc.dram_tensor("kp", [C, N], F32, kind="ExternalInput")
    o = nc.dram_tensor("o", [C, N], F32, kind="ExternalOutput")
    with tile.TileContext(nc) as tc:
        with tc.tile_pool(name="sb", bufs=2) as sb:
            t = sb.tile([P, 2, N], F32, tag="t")
            nc.sync.dma_start(t, qt.rearrange("(co p) n -> p co n", p=P))
            nc.sync.dma_start(o.rearrange("(co p) n -> p co n", p=P), t)
    nc.compile()
    return nc
